# revision 1
# baseline (speedup 1.0000x reference)
"""Trainium2 Bass kernel for ChemicalNet (per-species MLP / MoE routing).

Strategy
--------
Only atoms whose species is in {1, 6, 7, 8} produce output (others are 0),
and each such atom only needs ITS OWN species' 3-layer MLP.  The reference
runs all 4 expert networks on all atoms; we instead route on the host:

- host: map species -> expert index, collect per-expert atom index lists
- shard: 2 cores per expert, each core gets half of that expert's atoms
  (the per-core in_map carries that expert's weights, so the single SPMD
  program is expert-agnostic)
- host passes the gathered embedding columns TRANSPOSED ([128, n]) so the
  device needs no transposes: PE contracts over the partition axis directly
- device: L1 matmul+SiLU, L2 matmul(2-step accum)+SiLU, L3 matmul -> [1, n]
- host scatters the compact per-core outputs back to the full [N, 1] output

All shapes are compile-time constants derived from the actual input
(the Bass program is built fresh per call).
"""

import numpy as np

import concourse.bass as bass
import concourse.tile as tile
from concourse import bacc, mybir
from concourse.bass_utils import run_bass_kernel_spmd

N_CORES = 8
NSPECIES = 4
SPECIES_Z = np.array([1, 6, 7, 8], dtype=np.int32)
MAXIDX = 118
D = 128          # embedding dim
H = 256          # hidden dim
F = 512          # atom-chunk size (one PSUM bank of fp32)
FP = mybir.dt.float32


def _build_program(npad: int):
    """One SPMD program: a 3-layer per-expert MLP over `npad` atom columns."""
    nc = bacc.Bacc("TRN2", target_bir_lowering=False, debug=False,
                   num_devices=N_CORES)

    embT_d = nc.dram_tensor("embT", [D, npad], FP, kind="ExternalInput")
    w1_d = nc.dram_tensor("w1", [D, H], FP, kind="ExternalInput")
    w2_d = nc.dram_tensor("w2", [2, 128, H], FP, kind="ExternalInput")
    w3_d = nc.dram_tensor("w3", [128, 2], FP, kind="ExternalInput")
    b1_d = nc.dram_tensor("b1", [128, 2], FP, kind="ExternalInput")
    b2_d = nc.dram_tensor("b2", [128, 2], FP, kind="ExternalInput")
    b3_d = nc.dram_tensor("b3", [1, 1], FP, kind="ExternalInput")
    out_d = nc.dram_tensor("out", [1, npad], FP, kind="ExternalOutput")

    chunks = [(c0, min(F, npad - c0)) for c0 in range(0, npad, F)]

    with tile.TileContext(nc) as tc:
        with (
            tc.tile_pool(name="singles", bufs=1) as singles,
            tc.tile_pool(name="emb", bufs=4) as embp,
            tc.tile_pool(name="h1p", bufs=4) as h1p,
            tc.tile_pool(name="h2p", bufs=4) as h2p,
            tc.tile_pool(name="outp", bufs=3) as outp,
            tc.tile_pool(name="ps1", bufs=4, space="PSUM") as ps1p,
            tc.tile_pool(name="ps2", bufs=3, space="PSUM") as ps2p,
            tc.tile_pool(name="ps3", bufs=1, space="PSUM") as ps3p,
        ):
            w1_t = singles.tile([D, H], FP)
            nc.sync.dma_start(w1_t[:], w1_d[:])
            w2_t = singles.tile([128, 2 * H], FP)
            for r in range(2):
                nc.sync.dma_start(w2_t[:, r * H:(r + 1) * H], w2_d[r])
            w3_t = singles.tile([128, 2], FP)
            nc.sync.dma_start(w3_t[:], w3_d[:])
            b1_t = singles.tile([128, 2], FP)
            nc.sync.dma_start(b1_t[:], b1_d[:])
            b2_t = singles.tile([128, 2], FP)
            nc.sync.dma_start(b2_t[:], b2_d[:])
            b3_t = singles.tile([1, 1], FP)
            nc.sync.dma_start(b3_t[:], b3_d[:])

            for c0, f in chunks:
                emb_c = embp.tile([D, F], FP, tag="emb")
                nc.sync.dma_start(emb_c[:, :f], embT_d[:, c0:c0 + f])

                # L1: z1[m] = W1[:, m*128:...].T @ embT  -> SiLU
                h1 = []
                for m in range(2):
                    ps1 = ps1p.tile([128, F], FP, tag="ps1")
                    nc.tensor.matmul(ps1[:, :f],
                                     w1_t[:, m * 128:(m + 1) * 128],
                                     emb_c[:, :f], start=True, stop=True)
                    h1m = h1p.tile([128, F], FP, tag="h1")
                    nc.scalar.activation(h1m[:, :f], ps1[:, :f],
                                         mybir.ActivationFunctionType.Silu,
                                         bias=b1_t[:, m:m + 1])
                    h1.append(h1m)

                # L2: z2[m] = sum_r W2[r, :, m*128:...].T @ h1[r] -> SiLU
                h2 = []
                for m in range(2):
                    ps2 = ps2p.tile([128, F], FP, tag="ps2")
                    nc.tensor.matmul(ps2[:, :f],
                                     w2_t[:, 0 * H + m * 128:0 * H + (m + 1) * 128],
                                     h1[0][:, :f], start=True, stop=False)
                    nc.tensor.matmul(ps2[:, :f],
                                     w2_t[:, 1 * H + m * 128:1 * H + (m + 1) * 128],
                                     h1[1][:, :f], start=False, stop=True)
                    h2m = h2p.tile([128, F], FP, tag="h2")
                    nc.scalar.activation(h2m[:, :f], ps2[:, :f],
                                         mybir.ActivationFunctionType.Silu,
                                         bias=b2_t[:, m:m + 1])
                    h2.append(h2m)

                # L3: out = w3[:, 0].T @ h2[0] + w3[:, 1].T @ h2[1] + b3
                ps3 = ps3p.tile([1, F], FP, tag="ps3")
                nc.tensor.matmul(ps3[:, :f], w3_t[:, 0:1], h2[0][:, :f],
                                 start=True, stop=False)
                nc.tensor.matmul(ps3[:, :f], w3_t[:, 1:2], h2[1][:, :f],
                                 start=False, stop=True)
                out_t = outp.tile([1, F], FP, tag="out")
                nc.vector.tensor_scalar_add(out_t[:, :f], ps3[:, :f],
                                            b3_t[0:1, 0:1])
                nc.sync.dma_start(out_d[:, c0:c0 + f], out_t[:, :f])

    nc.compile()
    return nc


def _route(species: np.ndarray):
    """species values -> expert idx (-1 unknown); per-core row assignments."""
    conv = np.full(MAXIDX + 2, -1, dtype=np.int32)
    conv[SPECIES_Z] = np.arange(NSPECIES, dtype=np.int32)
    idx = conv[species]
    core_rows = []
    for s in range(NSPECIES):
        rows = np.flatnonzero(idx == s)
        h = (len(rows) + 1) // 2
        core_rows.append(rows[:h])
        core_rows.append(rows[h:])
    return core_rows


def _run(inputs: dict, trace: bool = False):
    species = inputs["species"]
    embedding = np.ascontiguousarray(inputs["embedding"], dtype=np.float32)
    n_atoms = species.shape[0]
    out_full = np.zeros((n_atoms, 1), dtype=np.float32)

    core_rows = _route(np.asarray(species))
    nmax = max(len(r) for r in core_rows)
    if nmax == 0:
        return out_full, None
    npad = -(-nmax // 4) * 4

    nc = _build_program(npad)

    in_maps = []
    for c in range(N_CORES):
        s = c // 2
        rows = core_rows[c]
        embT = np.zeros((D, npad), dtype=np.float32)
        if len(rows):
            embT[:, :len(rows)] = embedding[rows].T
        in_maps.append({
            "embT": embT,
            "w1": np.ascontiguousarray(inputs["W1"][s], dtype=np.float32),
            "w2": np.ascontiguousarray(
                np.asarray(inputs["W2"][s], dtype=np.float32).reshape(2, 128, H)),
            "w3": np.ascontiguousarray(
                np.asarray(inputs["W3"][s], dtype=np.float32).reshape(2, 128).T),
            "b1": np.ascontiguousarray(
                np.asarray(inputs["b1"][s], dtype=np.float32).reshape(2, 128).T),
            "b2": np.ascontiguousarray(
                np.asarray(inputs["b2"][s], dtype=np.float32).reshape(2, 128).T),
            "b3": np.asarray(inputs["b3"][s], dtype=np.float32).reshape(1, 1),
        })

    res = run_bass_kernel_spmd(nc, in_maps, core_ids=list(range(N_CORES)),
                               trace=trace)
    for c in range(N_CORES):
        rows = core_rows[c]
        if len(rows):
            out_full[rows, 0] = res.results[c]["out"][0, :len(rows)]
    return out_full, res


def kernel(**inputs) -> np.ndarray:
    out, _ = _run(inputs, trace=False)
    return out


# revision 2
# speedup vs baseline: 1.6840x; 1.6840x over previous
"""Trainium2 Bass kernel for ChemicalNet (per-species MLP / MoE routing).

Strategy
--------
Only atoms whose species is in {1, 6, 7, 8} produce output (others are 0),
and each such atom only needs ITS OWN species' 3-layer MLP.  The reference
runs all 4 expert networks on all atoms; we route on the host instead:

- host: map species -> expert index, collect per-expert atom index lists
- shard: 2 cores per expert, each core gets half of that expert's atoms
  (the per-core in_map carries that expert's weights, so the single SPMD
  program is expert-agnostic)
- host passes the gathered embedding columns TRANSPOSED ([128, n]) so the
  device needs no transposes: PE contracts over the partition axis directly
- device: L1 matmul+SiLU, L2 matmul (2-step K accum)+SiLU, L3 matmul -> [1,n]
- host scatters the compact per-core outputs back to the full [N, 1] output

Per-chunk (512 atoms) the two 128-row halves of the hidden layer land in one
[128, 1024] PSUM tile so a single ACTIVATE applies SiLU to both (the scalar
engine does not pipeline ACTIVATEs, so fewer/bigger is faster).  That merge
needs a bias that is constant along the free axis; biases in this problem
are identically zero, which the host verifies -- a nonzero-bias input takes
a (slower) per-half ACTIVATE path with per-partition bias.

All shapes are compile-time constants derived from the actual input
(the Bass program is built fresh per call).
"""

import numpy as np

import concourse.bass as bass
import concourse.tile as tile
from concourse import bacc, mybir
from concourse.bass_utils import run_bass_kernel_spmd

N_CORES = 8
NSPECIES = 4
SPECIES_Z = np.array([1, 6, 7, 8], dtype=np.int32)
MAXIDX = 118
D = 128          # embedding dim
H = 256          # hidden dim
F = 512          # atom-chunk size (one PSUM bank of fp32)
FP = mybir.dt.float32
SILU = mybir.ActivationFunctionType.Silu


def _build_program(npad: int, zero_bias: bool, mmdt):
    """One SPMD program: a 3-layer per-expert MLP over `npad` atom columns.

    mmdt: matmul operand dtype (float32, or float32r for ~2x PE throughput
    at ~1e-4 relative precision).
    """
    nc = bacc.Bacc("TRN2", target_bir_lowering=False, debug=False,
                   num_devices=N_CORES)

    embT_d = nc.dram_tensor("embT", [D, npad], mmdt, kind="ExternalInput")
    w1_d = nc.dram_tensor("w1", [D, H], mmdt, kind="ExternalInput")
    w2_d = nc.dram_tensor("w2", [2, 128, H], mmdt, kind="ExternalInput")
    w3_d = nc.dram_tensor("w3", [128, 2], mmdt, kind="ExternalInput")
    if not zero_bias:
        b1_d = nc.dram_tensor("b1", [128, 2], FP, kind="ExternalInput")
        b2_d = nc.dram_tensor("b2", [128, 2], FP, kind="ExternalInput")
        b3_d = nc.dram_tensor("b3", [1, 1], FP, kind="ExternalInput")
    out_d = nc.dram_tensor("out", [1, npad], FP, kind="ExternalOutput")

    chunks = [(c0, min(F, npad - c0)) for c0 in range(0, npad, F)]
    nch = len(chunks)

    with tile.TileContext(nc) as tc:
        with (
            tc.tile_pool(name="singles", bufs=1) as singles,
            tc.tile_pool(name="emb", bufs=nch) as embp,
            tc.tile_pool(name="z1p", bufs=nch) as z1p,
            tc.tile_pool(name="z2p", bufs=nch) as z2p,
            tc.tile_pool(name="outp", bufs=3) as outp,
            tc.tile_pool(name="ps", bufs=3, space="PSUM") as psp,
            tc.tile_pool(name="ps3", bufs=1, space="PSUM") as ps3p,
        ):
            # --- PE warm-up + ACT table preload while input DMAs run ---
            warm_w = singles.tile([128, 128], mmdt)
            nc.vector.memset(warm_w[:].bitcast(FP), 0.0)
            warm_x = singles.tile([128, F], mmdt)
            nc.vector.memset(warm_x[:].bitcast(FP), 0.0)
            warm_ps = psp.tile([128, 2 * F], FP, tag="ps", name="warm_ps")
            for i in range(5):
                nc.tensor.matmul(warm_ps[:, :F], warm_w[:], warm_x[:],
                                 start=True, stop=True)
            warm_act = singles.tile([128, 1], FP)
            nc.scalar.activation(warm_act[:], warm_w[:, 0:1].bitcast(FP), SILU)

            # --- weights on the gpsimd DMA queue (parallel with emb) ---
            w1_t = singles.tile([D, H], mmdt)
            nc.gpsimd.dma_start(w1_t[:], w1_d[:])
            w2_t = singles.tile([128, 2 * H], mmdt)
            for r in range(2):
                nc.gpsimd.dma_start(w2_t[:, r * H:(r + 1) * H], w2_d[r])
            w3_t = singles.tile([128, 2], mmdt)
            nc.gpsimd.dma_start(w3_t[:], w3_d[:])
            if not zero_bias:
                b1_t = singles.tile([128, 2], FP)
                nc.gpsimd.dma_start(b1_t[:], b1_d[:])
                b2_t = singles.tile([128, 2], FP)
                nc.gpsimd.dma_start(b2_t[:], b2_d[:])
                b3_t = singles.tile([1, 1], FP)
                nc.gpsimd.dma_start(b3_t[:], b3_d[:])

            emb_ts = []
            for ci, (c0, f) in enumerate(chunks):
                emb_c = embp.tile([D, F], mmdt, tag="emb", name=f"emb{ci}")
                nc.sync.dma_start(emb_c[:, :f], embT_d[:, c0:c0 + f])
                emb_ts.append(emb_c)

            def act_pair(z_t, ps_t, f, b_t):
                """SiLU both m-halves of a [128, 2*F] psum tile -> z SBUF."""
                if zero_bias:
                    if f == F:
                        nc.scalar.activation(z_t[:], ps_t[:], SILU)
                    else:
                        for m in range(2):
                            nc.scalar.activation(
                                z_t[:, m * F:m * F + f],
                                ps_t[:, m * F:m * F + f], SILU)
                else:
                    for m in range(2):
                        nc.scalar.activation(
                            z_t[:, m * F:m * F + f],
                            ps_t[:, m * F:m * F + f], SILU,
                            bias=b_t[:, m:m + 1])

            for ci, (c0, f) in enumerate(chunks):
                emb_c = emb_ts[ci]
                # L1: z1[m] = W1[:, m*128:(m+1)*128].T @ embT
                ps1 = psp.tile([128, 2 * F], FP, tag="ps", name=f"ps1_{ci}")
                for m in range(2):
                    nc.tensor.matmul(ps1[:, m * F:m * F + f],
                                     w1_t[:, m * 128:(m + 1) * 128],
                                     emb_c[:, :f], start=True, stop=True)
                z1 = z1p.tile([128, 2 * F], mmdt, tag="z1", name=f"z1_{ci}")
                act_pair(z1, ps1, f, None if zero_bias else b1_t)

                # L2: z2[m] = sum_r W2[r, :, m*128:(m+1)*128].T @ z1[r]
                ps2 = psp.tile([128, 2 * F], FP, tag="ps", name=f"ps2_{ci}")
                for m in range(2):
                    nc.tensor.matmul(ps2[:, m * F:m * F + f],
                                     w2_t[:, m * 128:m * 128 + 128],
                                     z1[:, :f], start=True, stop=False)
                    nc.tensor.matmul(ps2[:, m * F:m * F + f],
                                     w2_t[:, H + m * 128:H + m * 128 + 128],
                                     z1[:, F:F + f], start=False, stop=True)
                z2 = z2p.tile([128, 2 * F], mmdt, tag="z2", name=f"z2_{ci}")
                act_pair(z2, ps2, f, None if zero_bias else b2_t)

                # L3: out = w3[:, 0].T @ z2[m0] + w3[:, 1].T @ z2[m1] (+ b3)
                ps3 = ps3p.tile([1, F], FP, tag="ps3", name=f"ps3_{ci}")
                nc.tensor.matmul(ps3[:, :f], w3_t[:, 0:1], z2[:, :f],
                                 start=True, stop=False)
                nc.tensor.matmul(ps3[:, :f], w3_t[:, 1:2], z2[:, F:F + f],
                                 start=False, stop=True)
                out_t = outp.tile([1, F], FP, tag="out", name=f"out_{ci}")
                if zero_bias:
                    nc.vector.tensor_copy(out_t[:, :f], ps3[:, :f])
                else:
                    nc.vector.tensor_scalar_add(out_t[:, :f], ps3[:, :f],
                                                b3_t[0:1, 0:1])
                nc.sync.dma_start(out_d[:, c0:c0 + f], out_t[:, :f])

    nc.compile()
    return nc


def _route(species: np.ndarray):
    """species values -> expert idx (-1 unknown); per-core row assignments."""
    conv = np.full(MAXIDX + 2, -1, dtype=np.int32)
    conv[SPECIES_Z] = np.arange(NSPECIES, dtype=np.int32)
    idx = conv[species]
    core_rows = []
    for s in range(NSPECIES):
        rows = np.flatnonzero(idx == s)
        h = (len(rows) + 1) // 2
        core_rows.append(rows[:h])
        core_rows.append(rows[h:])
    return core_rows


def _run(inputs: dict, trace: bool = False, use_f32r: bool = False):
    species = inputs["species"]
    embedding = np.ascontiguousarray(inputs["embedding"], dtype=np.float32)
    n_atoms = species.shape[0]
    out_full = np.zeros((n_atoms, 1), dtype=np.float32)

    core_rows = _route(np.asarray(species))
    nmax = max(len(r) for r in core_rows)
    if nmax == 0:
        return out_full, None
    npad = -(-nmax // 4) * 4

    zero_bias = all(
        not np.any(np.asarray(inputs[k])) for k in ("b1", "b2", "b3"))
    mmdt = mybir.dt.float32r if use_f32r else FP
    nc = _build_program(npad, zero_bias, mmdt)

    in_maps = []
    for c in range(N_CORES):
        s = c // 2
        rows = core_rows[c]
        embT = np.zeros((D, npad), dtype=np.float32)
        if len(rows):
            embT[:, :len(rows)] = embedding[rows].T
        im = {
            "embT": embT,
            "w1": np.ascontiguousarray(inputs["W1"][s], dtype=np.float32),
            "w2": np.ascontiguousarray(
                np.asarray(inputs["W2"][s], dtype=np.float32).reshape(2, 128, H)),
            "w3": np.ascontiguousarray(
                np.asarray(inputs["W3"][s], dtype=np.float32).reshape(2, 128).T),
        }
        if not zero_bias:
            im["b1"] = np.ascontiguousarray(
                np.asarray(inputs["b1"][s], dtype=np.float32).reshape(2, 128).T)
            im["b2"] = np.ascontiguousarray(
                np.asarray(inputs["b2"][s], dtype=np.float32).reshape(2, 128).T)
            im["b3"] = np.asarray(inputs["b3"][s], dtype=np.float32).reshape(1, 1)
        in_maps.append(im)

    res = run_bass_kernel_spmd(nc, in_maps, core_ids=list(range(N_CORES)),
                               trace=trace)
    for c in range(N_CORES):
        rows = core_rows[c]
        if len(rows):
            out_full[rows, 0] = res.results[c]["out"][0, :len(rows)]
    return out_full, res


def kernel(**inputs) -> np.ndarray:
    out, _ = _run(inputs, trace=False)
    return out
